# revision 1
# baseline (speedup 1.0000x reference)
"""ConvNeXtSynthesisLayer Trainium2 kernel (8 NeuronCores, data-parallel over batch).

Self-contained: hardcodes shapes B=16, C=256, H=W=64, WD=512, K=7.
Each core processes 2 samples end-to-end on-chip:
  style affine (PE) -> depthwise 7x7 (DVE scalar_tensor_tensor MACs + PE diag-matmul
  row split) -> GroupNorm32 (accum_out sums + tiny group matmuls, folded into one
  ScalarE affine pass together with the style modulation) -> pwconv1 with
  demodulation+bias+GELU fused into the PSUM drain -> pwconv2 -> gamma*z + x.
"""

import os
import sys

sys.path.insert(0, "/opt/trn_rl_repo")

import numpy as np

import concourse.bass as bass
import concourse.tile as tile
from concourse import mybir
from concourse.bass_utils import run_bass_kernel_spmd


def _spill_multiwaits(ordered):
    """This walrus build accepts a single sync wait per instruction; move each
    extra wait onto an injected same-engine NoOp placed just before it."""
    for bb, insts in list(ordered.items()):
        out = []
        for inst in insts:
            si = getattr(inst, "sync_info", None)
            eng = getattr(inst, "engine", None)
            if si is not None and eng is not None and len(si.on_wait) > 1:
                waits = list(si.on_wait)
                for j, w in enumerate(waits[:-1]):
                    out.append(
                        mybir.InstNoOp(
                            name=f"{inst.name}-ws{j}",
                            engine=eng,
                            sync_info=mybir.SyncInfo(on_wait=[w], on_update=[]),
                            ins=[],
                            outs=[],
                        )
                    )
                inst.sync_info = mybir.SyncInfo(
                    on_wait=[waits[-1]], on_update=list(si.on_update)
                )
            out.append(inst)
        insts[:] = out


_OrigTCW = tile.TileClockWait


class _SpillTCW:
    def __init__(self, tc, ordered):
        self._inner = _OrigTCW(tc, ordered)
        self._tc = tc
        self._ordered = ordered

    def assign_waits(self, *a, **k):
        r = self._inner.assign_waits(*a, **k)
        _spill_multiwaits(self._ordered)
        return r

    def add_sem_waits(self, raw_inst, *a, **k):
        # kernel-tail drain: split a multi-wait drain into single-wait drains
        # (order-insensitive — the all-engine barrier follows them all)
        r = self._inner.add_sem_waits(raw_inst, *a, **k)
        si = getattr(raw_inst, "sync_info", None)
        if si is not None and len(si.on_wait) > 1:
            waits = list(si.on_wait)
            raw_inst.sync_info = mybir.SyncInfo(
                on_wait=waits[:1], on_update=list(si.on_update)
            )
            for w in waits[1:]:
                d = self._tc.nc.sync.drain()
                d.ins.sync_info = mybir.SyncInfo(on_wait=[w], on_update=[])
        return r

    def __getattr__(self, k):
        return getattr(self._inner, k)


tile.TileClockWait = _SpillTCW

F32 = mybir.dt.float32
BF16 = mybir.dt.bfloat16
AOP = mybir.AluOpType
ACT = mybir.ActivationFunctionType

B, C, H, W = 16, 256, 64, 64
WD, K7 = 512, 7
NCORES = 8
BLOC = B // NCORES          # samples per core = 2
CH = C // 128               # channel chunks = 2
HW = H * W                  # 4096
NBLK = 8                    # pwconv pixel blocks of 512
BLKN = HW // NBLK           # 512
HP, WP = 70, 72             # padded image (3 rows top/bot; cols: data at 4+j / 5+j)

# dwconv row split: rows [0, DVE_ROWS) on VectorE, rest on TensorE diag-matmuls
PE_ROWS = int(os.environ.get("KERNEL_PE_ROWS", "24"))
assert PE_ROWS % 8 == 0 and 0 <= PE_ROWS <= 64
DVE_ROWS = 64 - PE_ROWS
NPEBLK = PE_ROWS // 8
NPART = 1 + NPEBLK          # per-channel sum partials (1 DVE + per PE block)

TAPS = [(dy, dx) for dy in range(K7) for dx in range(K7)]


def _tap_src(xpe, xpo, dy, dx, r0, nrows):
    """AP reading x[c, i+dy-3, j+dx-3] for output rows i in [r0, r0+nrows), all j.

    xpe holds data at column 4+j, xpo at 5+j; picks the copy whose read offset is
    even so the DVE 2x packed mode engages.
    """
    if dx % 2 == 1:
        return xpe[:, r0 + dy : r0 + dy + nrows, 1 + dx : 1 + dx + 64]
    return xpo[:, r0 + dy : r0 + dy + nrows, 2 + dx : 2 + dx + 64]


def build_nc():
    nc = bass.Bass()

    # ---- DRAM I/O (per-core shards; weights replicated) ----
    x4 = nc.dram_tensor("x4", [BLOC, CH, 128, HW], F32, kind="ExternalInput")
    wt = nc.dram_tensor("wt", [128, BLOC, 4], F32, kind="ExternalInput")
    aff = nc.dram_tensor("aff", [128, 4, 3 * C], F32, kind="ExternalInput")
    affb = nc.dram_tensor("affb", [128, 6], F32, kind="ExternalInput")
    dww = nc.dram_tensor("dww", [128, CH * 49], F32, kind="ExternalInput")
    dwb = nc.dram_tensor("dwb", [128, CH], F32, kind="ExternalInput")
    ngt = nc.dram_tensor("ngt", [128, CH], F32, kind="ExternalInput")
    nbt = nc.dram_tensor("nbt", [128, CH], F32, kind="ExternalInput")
    p1t = nc.dram_tensor("p1t", [128, CH, 4 * C], F32, kind="ExternalInput")
    p1b = nc.dram_tensor("p1b", [128, 8], F32, kind="ExternalInput")
    p2t = nc.dram_tensor("p2t", [128, 8, C], F32, kind="ExternalInput")
    p2b = nc.dram_tensor("p2b", [128, CH], F32, kind="ExternalInput")
    gam = nc.dram_tensor("gam", [128, CH], F32, kind="ExternalInput")
    idm = nc.dram_tensor("idm", [128, 128], F32, kind="ExternalInput")
    gmat = nc.dram_tensor("gmat", [128, 16], F32, kind="ExternalInput")
    gmt = nc.dram_tensor("gmt", [16, 128], F32, kind="ExternalInput")
    out4 = nc.dram_tensor("out4", [BLOC, CH, 128, HW], F32, kind="ExternalOutput")

    with tile.TileContext(nc) as tc:
        from contextlib import ExitStack

        with ExitStack() as ctx:
            consts = ctx.enter_context(tc.tile_pool(name="consts", bufs=1))
            wstage = ctx.enter_context(tc.tile_pool(name="wstage", bufs=1))
            xp = ctx.enter_context(tc.tile_pool(name="xp", bufs=2))
            xpadp = ctx.enter_context(tc.tile_pool(name="xpadp", bufs=2))
            yp = ctx.enter_context(tc.tile_pool(name="yp", bufs=1))
            dwaccp = ctx.enter_context(tc.tile_pool(name="dwaccp", bufs=2))
            zp = ctx.enter_context(tc.tile_pool(name="zp", bufs=12))
            tfp = ctx.enter_context(tc.tile_pool(name="tfp", bufs=2))
            osp = ctx.enter_context(tc.tile_pool(name="osp", bufs=3))
            xrp = ctx.enter_context(tc.tile_pool(name="xrp", bufs=4))
            smallp = ctx.enter_context(tc.tile_pool(name="smallp", bufs=2))
            ps1 = ctx.enter_context(tc.tile_pool(name="ps1", bufs=2, space="PSUM"))
            ps2 = ctx.enter_context(tc.tile_pool(name="ps2", bufs=2, space="PSUM"))
            psdw = ctx.enter_context(tc.tile_pool(name="psdw", bufs=2, space="PSUM"))
            psm = ctx.enter_context(tc.tile_pool(name="psm", bufs=2, space="PSUM"))

            # ---- load constants ----
            aff_s = consts.tile([128, 4, 3 * C], F32)
            nc.sync.dma_start(out=aff_s[:], in_=aff[:])
            wt_s = consts.tile([128, BLOC, 4], F32)
            nc.sync.dma_start(out=wt_s[:], in_=wt[:])
            affb_s = consts.tile([128, 6], F32)
            nc.sync.dma_start(out=affb_s[:], in_=affb[:])
            dww_s = consts.tile([128, CH * 49], F32)
            nc.sync.dma_start(out=dww_s[:], in_=dww[:])
            dwb_s = consts.tile([128, CH], F32)
            nc.sync.dma_start(out=dwb_s[:], in_=dwb[:])
            ng_s = consts.tile([128, CH], F32)
            nc.sync.dma_start(out=ng_s[:], in_=ngt[:])
            nb_s = consts.tile([128, CH], F32)
            nc.sync.dma_start(out=nb_s[:], in_=nbt[:])
            p1b_s = consts.tile([128, 8], F32)
            nc.sync.dma_start(out=p1b_s[:], in_=p1b[:])
            p2b_s = consts.tile([128, CH], F32)
            nc.sync.dma_start(out=p2b_s[:], in_=p2b[:])
            gam_s = consts.tile([128, CH], F32)
            nc.sync.dma_start(out=gam_s[:], in_=gam[:])
            gmat_s = consts.tile([128, 16], F32)
            nc.sync.dma_start(out=gmat_s[:], in_=gmat[:])
            gmt_s = consts.tile([16, 128], F32)
            nc.sync.dma_start(out=gmt_s[:], in_=gmt[:])

            # staged fp32 weights -> bf16
            p1t_f = wstage.tile([128, CH, 4 * C], F32, tag="wstage")
            nc.sync.dma_start(out=p1t_f[:], in_=p1t[:])
            p1t_b = consts.tile([128, CH, 4 * C], BF16)
            nc.vector.tensor_copy(out=p1t_b[:], in_=p1t_f[:])
            p2t_f = wstage.tile([128, 8, C], F32, tag="wstage")
            nc.sync.dma_start(out=p2t_f[:], in_=p2t[:])
            p2t_b = consts.tile([128, 8, C], BF16)
            nc.vector.tensor_copy(out=p2t_b[:], in_=p2t_f[:])
            idm_f = wstage.tile([128, 128], F32, tag="wstage")
            nc.sync.dma_start(out=idm_f[:], in_=idm[:])
            idm_b = consts.tile([128, 128], BF16)
            nc.vector.tensor_copy(out=idm_b[:], in_=idm_f[:])

            p1sq_b = consts.tile([128, CH, 4 * C], BF16)
            nc.scalar.square(out=p1sq_b[:], in_=p1t_b[:])
            gb_s = consts.tile([128, CH], F32)
            nc.vector.tensor_mul(out=gb_s[:], in0=gam_s[:], in1=p2b_s[:])
            dwbsq_s = consts.tile([128, CH], F32)
            nc.vector.tensor_mul(out=dwbsq_s[:], in0=dwb_s[:], in1=dwb_s[:])
            eps8 = consts.tile([128, 1], F32)
            nc.vector.memset(eps8[:], 1e-8)
            eps5 = consts.tile([128, 1], F32)
            nc.vector.memset(eps5[:], 1e-5)

            # diag weight matrices for PE taps: dg[:, ch, t, :] = diag(dw[ch, t])
            if NPEBLK > 0:
                dg = consts.tile([128, CH, 49, 128], BF16)
                for ch in range(CH):
                    for t in range(49):
                        nc.vector.tensor_scalar_mul(
                            out=dg[:, ch, t, :],
                            in0=idm_b[:],
                            scalar1=dww_s[:, ch * 49 + t : ch * 49 + t + 1],
                        )

            # ---- engine sem pre-touches: this walrus accepts only ONE sync wait
            # per instruction, so each engine absorbs every const-DMA semaphore
            # via tiny reads before real work (one fresh sem per op thereafter)
            probe = consts.tile([128, 4], F32)
            for i_, t_ in enumerate([dww_s, dwb_s, ng_s, nb_s, gam_s, p2b_s, affb_s, p1b_s]):
                nc.vector.tensor_copy(out=probe[0:1, 0:1], in_=t_[0:1, 0:1])
            for i_, t_ in enumerate([p1b_s, gam_s, dwb_s]):
                nc.scalar.copy(out=probe[0:1, 1:2], in_=t_[0:1, 0:1])

            # ---- PE warmup touches: absorb one fresh semaphore each so no real
            # matmul needs >1 sync wait (walrus LDWEIGHTS has a single wait slot)
            warm = psm.tile([2, 2], F32, tag="misc")
            touch = [aff_s, wt_s, gmat_s, gmt_s, p1sq_b, p1t_b, p2t_b]
            if NPEBLK > 0:
                touch.append(dg)
            for tt_ in touch:
                sl2 = tuple([slice(0, 2)] + [0] * (len(tt_[:].shape) - 2) + [slice(0, 2)])
                ap2 = tt_[sl2] if len(tt_[:].shape) > 2 else tt_[0:2, 0:2]
                nc.tensor.matmul(warm[:], ap2, ap2, start=True, stop=True)

            # ---- style affine for both samples: s = aff_w @ w_b + aff_b ----
            psty = psm.tile([128, 6, BLOC], F32, tag="misc")
            for m in range(6):
                for k in range(4):
                    nc.tensor.matmul(
                        psty[:, m, :],
                        aff_s[:, k, m * 128 : (m + 1) * 128],
                        wt_s[:, :, k],
                        start=(k == 0),
                        stop=(k == 3),
                    )
            s_s = consts.tile([128, 6, BLOC], F32)
            for b in range(BLOC):
                nc.vector.tensor_add(out=s_s[:, :, b], in0=psty[:, :, b], in1=affb_s[:])
            # style = s1*s2 + s3 ; layout stl[:, ch*BLOC + b]
            stl = consts.tile([128, CH * BLOC], F32)
            tmp22 = consts.tile([128, CH, BLOC], F32)
            for b in range(BLOC):
                nc.vector.tensor_mul(
                    out=tmp22[:, :, b], in0=s_s[:, 0:2, b], in1=s_s[:, 2:4, b]
                )
                for ch in range(CH):
                    nc.vector.tensor_add(
                        out=stl[:, ch * BLOC + b : ch * BLOC + b + 1],
                        in0=tmp22[:, ch, b : b + 1],
                        in1=s_s[:, 4 + ch, b : b + 1],
                    )
            stlsq_b = consts.tile([128, CH * BLOC], BF16)
            nc.scalar.square(out=stlsq_b[:], in_=stl[:])

            # ---- dcoef: rsqrt(pw1_w^2 @ style^2 + 1e-8) ; layout dco[:, o*BLOC+b]
            psd = psm.tile([128, 8, BLOC], F32, tag="misc")
            for o in range(8):
                for i in range(CH):
                    nc.tensor.matmul(
                        psd[:, o, :],
                        p1sq_b[:, i, o * 128 : (o + 1) * 128],
                        stlsq_b[:, i * BLOC : (i + 1) * BLOC],
                        start=(i == 0),
                        stop=(i == CH - 1),
                    )
            dct = consts.tile([128, 8 * BLOC], F32)
            nc.scalar.activation(
                out=dct[:].rearrange("p (o b) -> p o b", b=BLOC),
                in_=psd[:],
                func=ACT.Sqrt,
                bias=eps8[:],
            )
            dco = consts.tile([128, 8 * BLOC], F32)
            nc.vector.reciprocal(out=dco[:], in_=dct[:])

            # ---- main per-sample pipeline ----
            for b in range(BLOC):
                y_s = yp.tile([128, CH, HW], BF16)
                sums = smallp.tile([128, CH * NPART], F32, tag="sums")
                ysq = smallp.tile([128, CH], F32, tag="ysq")

                for ch in range(CH):
                    x_s = xp.tile([128, HW], F32, tag="x")
                    nc.sync.dma_start(out=x_s[:], in_=x4[b, ch])
                    xpe = xpadp.tile([128, HP, WP], BF16, tag="xpe")
                    xpo = xpadp.tile([128, HP, WP], BF16, tag="xpo")
                    nc.vector.memset(xpe[:], 0.0)
                    nc.vector.memset(xpo[:], 0.0)
                    xv = x_s[:].rearrange("p (h w) -> p h w", w=64)
                    nc.vector.tensor_copy(out=xpe[:, 3:67, 4:68], in_=xv)
                    nc.vector.tensor_copy(out=xpo[:, 3:67, 5:69], in_=xv)

                    # --- depthwise conv: DVE rows ---
                    if DVE_ROWS > 0:
                        pa = dwaccp.tile([128, DVE_ROWS, 64], BF16, tag="dwacc")
                        pb = dwaccp.tile([128, DVE_ROWS, 64], BF16, tag="dwacc")
                        ydve = y_s[:, ch, 0 : DVE_ROWS * 64].rearrange(
                            "p (h w) -> p h w", w=64
                        )
                        cur, nxt = pa, pb
                        for t, (dy, dx) in enumerate(TAPS):
                            src = _tap_src(xpe, xpo, dy, dx, 0, DVE_ROWS)
                            sc = dww_s[:, ch * 49 + t : ch * 49 + t + 1]
                            if t == 0:
                                nc.vector.tensor_scalar_mul(
                                    out=cur[:], in0=src, scalar1=sc
                                )
                            elif t < 48:
                                nc.vector.scalar_tensor_tensor(
                                    out=nxt[:],
                                    in0=src,
                                    scalar=sc,
                                    in1=cur[:],
                                    op0=AOP.mult,
                                    op1=AOP.add,
                                )
                                cur, nxt = nxt, cur
                            else:
                                nc.vector.scalar_tensor_tensor(
                                    out=ydve,
                                    in0=src,
                                    scalar=sc,
                                    in1=cur[:],
                                    op0=AOP.mult,
                                    op1=AOP.add,
                                    accum_out=sums[:, ch * NPART : ch * NPART + 1],
                                )

                    # --- depthwise conv: PE rows (diag matmuls into PSUM) ---
                    if NPEBLK > 0:
                        wps = psm.tile([2, 2], F32, tag="misc", name="wps")
                        nc.tensor.matmul(wps[:], xpe[0:2, 0, 0:2], xpe[0:2, 0, 0:2],
                                         start=True, stop=True)
                        nc.tensor.matmul(wps[:], xpo[0:2, 0, 0:2], xpo[0:2, 0, 0:2],
                                         start=True, stop=True)
                    for blk in range(NPEBLK):
                        r0 = DVE_ROWS + blk * 8
                        pdw = psdw.tile([128, 8, 64], F32, tag="dw")
                        for t, (dy, dx) in enumerate(TAPS):
                            nc.tensor.matmul(
                                pdw[:],
                                dg[:, ch, t, :],
                                _tap_src(xpe, xpo, dy, dx, r0, 8),
                                start=(t == 0),
                                stop=(t == 48),
                            )
                        nc.scalar.activation(
                            out=y_s[:, ch, r0 * 64 : (r0 + 8) * 64].rearrange(
                                "p (h w) -> p h w", w=64
                            ),
                            in_=pdw[:],
                            func=ACT.Copy,
                            accum_out=sums[
                                :, ch * NPART + 1 + blk : ch * NPART + 2 + blk
                            ],
                        )

                    # --- sum of y^2 for group stats (dummy streaming out into xpo,
                    # which is dead after the taps) ---
                    nc.scalar.activation(
                        out=xpo[:].rearrange("p a c -> p (a c)")[:, 0:HW],
                        in_=y_s[:, ch, :],
                        func=ACT.Square,
                        accum_out=ysq[:, ch : ch + 1],
                    )

                # --- GroupNorm stats (32 groups of 8 channels) ---
                stats_c = smallp.tile([128, 4], F32, tag="stats")
                # per-channel conv sums
                sc_sum = smallp.tile([128, CH], F32, tag="scs")
                nc.vector.tensor_reduce(
                    out=sc_sum[:],
                    in_=sums[:].rearrange("p (c k) -> p c k", k=NPART),
                    axis=mybir.AxisListType.X,
                    op=AOP.add,
                )
                # adjust for dw bias: s' = s + 4096*b ; q' = q + 2*b*s + 4096*b^2
                nc.vector.scalar_tensor_tensor(
                    out=stats_c[:, 0:2],
                    in0=dwb_s[:],
                    scalar=float(HW),
                    in1=sc_sum[:],
                    op0=AOP.mult,
                    op1=AOP.add,
                )
                t_bs = smallp.tile([128, CH], F32, tag="tbs")
                nc.vector.tensor_mul(out=t_bs[:], in0=dwb_s[:], in1=sc_sum[:])
                t_q1 = smallp.tile([128, CH], F32, tag="tq1")
                nc.vector.scalar_tensor_tensor(
                    out=t_q1[:],
                    in0=t_bs[:],
                    scalar=2.0,
                    in1=ysq[:],
                    op0=AOP.mult,
                    op1=AOP.add,
                )
                nc.vector.scalar_tensor_tensor(
                    out=stats_c[:, 2:4],
                    in0=dwbsq_s[:],
                    scalar=float(HW),
                    in1=t_q1[:],
                    op0=AOP.mult,
                    op1=AOP.add,
                )
                gps = psm.tile([16, 4], F32, tag="misc")
                nc.tensor.matmul(gps[:], gmat_s[:], stats_c[:], start=True, stop=True)
                gsb = smallp.tile([16, 4], F32, tag="gsb")
                nc.vector.tensor_copy(out=gsb[:], in_=gps[:])
                grp4 = smallp.tile([16, 4], F32, tag="grp4")
                n_per_group = 8 * HW  # 32768
                nc.vector.tensor_scalar_mul(
                    out=grp4[:, 0:2], in0=gsb[:, 0:2], scalar1=1.0 / n_per_group
                )
                msq = smallp.tile([16, 2], F32, tag="msq")
                nc.vector.tensor_scalar_mul(
                    out=msq[:], in0=gsb[:, 2:4], scalar1=1.0 / n_per_group
                )
                mg2 = smallp.tile([16, 2], F32, tag="mg2")
                nc.vector.tensor_mul(out=mg2[:], in0=grp4[:, 0:2], in1=grp4[:, 0:2])
                var_t = smallp.tile([16, 2], F32, tag="var")
                nc.vector.tensor_sub(out=var_t[:], in0=msq[:], in1=mg2[:])
                sd_t = smallp.tile([16, 2], F32, tag="sd")
                nc.scalar.activation(out=sd_t[:], in_=var_t[:], func=ACT.Sqrt, bias=eps5[0:16, :])
                nc.vector.reciprocal(out=grp4[:, 2:4], in_=sd_t[:])
                bps = psm.tile([128, 4], F32, tag="misc")
                nc.tensor.matmul(bps[:], gmt_s[:], grp4[:], start=True, stop=True)
                mrc = smallp.tile([128, 4], F32, tag="mrc")
                nc.vector.tensor_copy(out=mrc[:], in_=bps[:])

                # per-channel affine A*y + B folding groupnorm affine, style, dw bias
                abf = smallp.tile([128, 4], F32, tag="abf")  # [A0 A1 B0 B1]
                a0t = smallp.tile([128, 2], F32, tag="a0t")
                for ch in range(CH):
                    stl_c = stl[:, ch * BLOC + b : ch * BLOC + b + 1]
                    nc.vector.tensor_mul(
                        out=a0t[:, ch : ch + 1],
                        in0=ng_s[:, ch : ch + 1],
                        in1=mrc[:, 2 + ch : 3 + ch],
                    )
                    nc.vector.tensor_mul(
                        out=abf[:, ch : ch + 1], in0=a0t[:, ch : ch + 1], in1=stl_c
                    )
                    t2 = smallp.tile([128, 1], F32, tag="t2")
                    nc.vector.tensor_mul(
                        out=t2[:], in0=mrc[:, ch : ch + 1], in1=a0t[:, ch : ch + 1]
                    )
                    t3 = smallp.tile([128, 1], F32, tag="t3")
                    nc.vector.tensor_sub(out=t3[:], in0=nb_s[:, ch : ch + 1], in1=t2[:])
                    t4 = smallp.tile([128, 1], F32, tag="t4")
                    nc.vector.tensor_mul(out=t4[:], in0=t3[:], in1=stl_c)
                    nc.vector.scalar_tensor_tensor(
                        out=abf[:, 2 + ch : 3 + ch],
                        in0=abf[:, ch : ch + 1],
                        scalar=dwb_s[:, ch : ch + 1],
                        in1=t4[:],
                        op0=AOP.mult,
                        op1=AOP.add,
                    )

                # modulated-normalized activations, in place on y
                for ch in range(CH):
                    nc.scalar.activation(
                        out=y_s[:, ch, :],
                        in_=y_s[:, ch, :],
                        func=ACT.Lrelu,
                        bias=abf[:, 2 + ch : 3 + ch],
                        scale=abf[:, ch : ch + 1],
                        alpha=1.0,
                    )

                # --- pwconv1 -> gelu -> pwconv2 -> gamma*z + x, per 512-px block ---
                for blk in range(NBLK):
                    sl = slice(blk * BLKN, (blk + 1) * BLKN)
                    zg = [zp.tile([128, BLKN], BF16, tag="zg", name=f"zg{o}") for o in range(8)]
                    for o in range(8):
                        pz = ps1.tile([128, BLKN], F32, tag="pz")
                        for i in range(CH):
                            nc.tensor.matmul(
                                pz[:],
                                p1t_b[:, i, o * 128 : (o + 1) * 128],
                                y_s[:, i, sl],
                                start=(i == 0),
                                stop=(i == CH - 1),
                            )
                        nc.scalar.activation(
                            out=zg[o][:],
                            in_=pz[:],
                            func=ACT.Gelu,
                            bias=p1b_s[:, o : o + 1],
                            scale=dco[:, o * BLOC + b : o * BLOC + b + 1],
                        )
                    for c in range(CH):
                        p2ps = ps2.tile([128, BLKN], F32, tag="p2")
                        for o in range(8):
                            nc.tensor.matmul(
                                p2ps[:],
                                p2t_b[:, o, c * 128 : (c + 1) * 128],
                                zg[o][:],
                                start=(o == 0),
                                stop=(o == 7),
                            )
                        tf = tfp.tile([128, BLKN], F32, tag="tf")
                        nc.scalar.activation(
                            out=tf[:],
                            in_=p2ps[:],
                            func=ACT.Lrelu,
                            bias=gb_s[:, c : c + 1],
                            scale=gam_s[:, c : c + 1],
                            alpha=1.0,
                        )
                        xr = xrp.tile([128, BLKN], F32, tag="xr")
                        nc.sync.dma_start(out=xr[:], in_=x4[b, c, :, sl])
                        nc.vector.tensor_copy(out=probe[0:1, 2:3], in_=xr[0:1, 0:1])
                        ost = osp.tile([128, BLKN], F32, tag="os")
                        nc.vector.tensor_add(out=ost[:], in0=tf[:], in1=xr[:])
                        nc.sync.dma_start(out=out4[b, c, :, sl], in_=ost[:])

    return nc


_NC = None


def _get_nc():
    global _NC
    if _NC is None:
        _NC = build_nc()
    return _NC


def _prep_maps(x, w, aff_w, aff_b, dw_w, dw_b, norm_g, norm_b, pw1_w, pw1_b, pw2_w,
               pw2_b, gamma):
    f = np.float32
    ct = lambda a: np.ascontiguousarray(a, dtype=f)
    common = {
        "aff": ct(aff_w.T.reshape(4, 128, 3 * C).transpose(1, 0, 2)),
        "affb": ct(aff_b.reshape(6, 128).T),
        "dww": ct(dw_w.reshape(C, 49).reshape(CH, 128, 49).transpose(1, 0, 2)
                  .reshape(128, CH * 49)),
        "dwb": ct(dw_b.reshape(CH, 128).T),
        "ngt": ct(norm_g.reshape(CH, 128).T),
        "nbt": ct(norm_b.reshape(CH, 128).T),
        "p1t": ct(pw1_w.T.reshape(CH, 128, 4 * C).transpose(1, 0, 2)),
        "p1b": ct(pw1_b.reshape(8, 128).T),
        "p2t": ct(pw2_w.T.reshape(8, 128, C).transpose(1, 0, 2)),
        "p2b": ct(pw2_b.reshape(CH, 128).T),
        "gam": ct(gamma.reshape(CH, 128).T),
        "idm": np.eye(128, dtype=f),
        "gmat": ct((np.arange(128)[:, None] // 8 == np.arange(16)[None, :])),
        "gmt": ct((np.arange(16)[:, None] == np.arange(128)[None, :] // 8)),
    }
    in_maps = []
    for i in range(NCORES):
        sl = slice(i * BLOC, (i + 1) * BLOC)
        m = dict(common)
        m["x4"] = ct(x[sl].reshape(BLOC, CH, 128, HW))
        m["wt"] = ct(w[sl].reshape(BLOC, 4, 128).transpose(2, 0, 1))
        in_maps.append(m)
    return in_maps


LAST_EXEC_NS = None


def _run(inputs, trace=False):
    global LAST_EXEC_NS
    nc = _get_nc()
    in_maps = _prep_maps(**inputs)
    res = run_bass_kernel_spmd(nc, in_maps, core_ids=list(range(NCORES)), trace=trace)
    LAST_EXEC_NS = res.exec_time_ns
    outs = [res.results[i]["out4"].reshape(BLOC, C, H, W) for i in range(NCORES)]
    return np.concatenate(outs, axis=0).astype(np.float32)


def kernel(**inputs):
    return _run({k: np.asarray(v) for k, v in inputs.items()}, trace=False)



# revision 2
# speedup vs baseline: 2.2987x; 2.2987x over previous
"""ConvNeXtSynthesisLayer Trainium2 kernel (8 NeuronCores, data-parallel over batch).

Self-contained: hardcodes shapes B=16, C=256, H=W=64, WD=512, K=7.
Each core processes 2 samples end-to-end on-chip. v2: the depthwise 7x7 conv and
both pointwise convs run on the PE in fp8 DoubleRow mode (K=256 per matmul; the
dwconv pairs two taps per matmul via a hand-built 2-k-tile access pattern with
even byte deltas). GroupNorm stats come from the PSUM drains (Copy+Square with
accum_out); the per-channel affine (GroupNorm x style modulation) is folded into
one DVE tensor_scalar pass; demodulation and fp8 weight pre-scales fold into the
ScalarE drain scales.
"""

import os
import sys

sys.path.insert(0, "/opt/trn_rl_repo")

import numpy as np

import concourse.bass as bass
import concourse.tile as tile
from concourse import mybir
from concourse.bass_utils import run_bass_kernel_spmd


def _spill_multiwaits(ordered):
    """This walrus build accepts a single sync wait per instruction; move each
    extra wait onto an injected same-engine NoOp placed just before it."""
    for bb, insts in list(ordered.items()):
        out = []
        for inst in insts:
            si = getattr(inst, "sync_info", None)
            eng = getattr(inst, "engine", None)
            if si is not None and eng is not None and len(si.on_wait) > 1:
                waits = list(si.on_wait)
                for j, w in enumerate(waits[:-1]):
                    out.append(
                        mybir.InstNoOp(
                            name=f"{inst.name}-ws{j}",
                            engine=eng,
                            sync_info=mybir.SyncInfo(on_wait=[w], on_update=[]),
                            ins=[],
                            outs=[],
                        )
                    )
                inst.sync_info = mybir.SyncInfo(
                    on_wait=[waits[-1]], on_update=list(si.on_update)
                )
            out.append(inst)
        insts[:] = out


_OrigTCW = tile.TileClockWait


class _SpillTCW:
    def __init__(self, tc, ordered):
        self._inner = _OrigTCW(tc, ordered)
        self._tc = tc
        self._ordered = ordered

    def assign_waits(self, *a, **k):
        r = self._inner.assign_waits(*a, **k)
        _spill_multiwaits(self._ordered)
        return r

    def add_sem_waits(self, raw_inst, *a, **k):
        # kernel-tail drain: split a multi-wait drain into single-wait drains
        # (order-insensitive — the all-engine barrier follows them all)
        r = self._inner.add_sem_waits(raw_inst, *a, **k)
        si = getattr(raw_inst, "sync_info", None)
        if si is not None and len(si.on_wait) > 1:
            waits = list(si.on_wait)
            raw_inst.sync_info = mybir.SyncInfo(
                on_wait=waits[:1], on_update=list(si.on_update)
            )
            for w in waits[1:]:
                d = self._tc.nc.sync.drain()
                d.ins.sync_info = mybir.SyncInfo(on_wait=[w], on_update=[])
        return r

    def __getattr__(self, k):
        return getattr(self._inner, k)


tile.TileClockWait = _SpillTCW

F32 = mybir.dt.float32
BF16 = mybir.dt.bfloat16
FP8 = mybir.dt.float8e4
AOP = mybir.AluOpType
ACT = mybir.ActivationFunctionType
PM = mybir.MatmulPerfMode

B, C, H, W = 16, 256, 64, 64
WD, K7 = 512, 7
NCORES = 8
BLOC = B // NCORES          # samples per core = 2
CH = C // 128               # channel chunks = 2
HW = H * W                  # 4096
NBLK = 8                    # pwconv pixel blocks of 512
BLKN = HW // NBLK           # 512
HP, WP = 71, 72             # padded image (3 top, 4 bottom junk; data at col 3+j)

SCL_DW = 64.0               # fp8 pre-scale of depthwise taps
SCL_P1 = 32.0               # fp8 pre-scale of pwconv1 weights
SCL_P2 = 32.0               # fp8 pre-scale of pwconv2 weights

# tap pairs with EVEN flat-offset deltas (DoubleRow k-tile stride must be
# 2-byte aligned); offsets are dy*72+dx in the padded image
PAIRS = []
for dy in range(7):
    PAIRS.append((dy * 7 + 0, dy * 7 + 2))
    PAIRS.append((dy * 7 + 4, dy * 7 + 6))
    PAIRS.append((dy * 7 + 1, dy * 7 + 3))
for dy in range(0, 6, 2):
    PAIRS.append((dy * 7 + 5, (dy + 1) * 7 + 5))
PAIRS.append((6 * 7 + 5, None))
assert len(PAIRS) == 25
NPAIR = 25


def _toff(t):
    return (t // 7) * 72 + (t % 7)


# 7-row output blocks in padded geometry (last block 1 row)
ROWBLKS = [(r, min(7, 64 - r)) for r in range(0, 64, 7)]
NRB = len(ROWBLKS)          # 10
# psum-bank groups for stationary-weight reuse
RBGROUPS = [ROWBLKS[0:3], ROWBLKS[3:6], ROWBLKS[6:9], ROWBLKS[9:10]]


def build_nc():
    nc = bass.Bass()

    # ---- DRAM I/O (per-core shards; weights replicated) ----
    x4 = nc.dram_tensor("x4", [BLOC, CH, 128, HW], F32, kind="ExternalInput")
    wt = nc.dram_tensor("wt", [128, BLOC, 4], F32, kind="ExternalInput")
    aff = nc.dram_tensor("aff", [128, 4, 3 * C], F32, kind="ExternalInput")
    affb = nc.dram_tensor("affb", [128, 6], F32, kind="ExternalInput")
    dww = nc.dram_tensor("dww", [128, CH * 49], F32, kind="ExternalInput")
    dwb = nc.dram_tensor("dwb", [128, CH], F32, kind="ExternalInput")
    ngt = nc.dram_tensor("ngt", [128, CH], F32, kind="ExternalInput")
    nbt = nc.dram_tensor("nbt", [128, CH], F32, kind="ExternalInput")
    p1t = nc.dram_tensor("p1t", [128, CH, 4 * C], F32, kind="ExternalInput")
    p1b = nc.dram_tensor("p1b", [128, 8], F32, kind="ExternalInput")
    p2t = nc.dram_tensor("p2t", [128, 8, C], F32, kind="ExternalInput")
    p2b = nc.dram_tensor("p2b", [128, CH], F32, kind="ExternalInput")
    gam = nc.dram_tensor("gam", [128, CH], F32, kind="ExternalInput")
    idm = nc.dram_tensor("idm", [128, 128], F32, kind="ExternalInput")
    gmat = nc.dram_tensor("gmat", [128, 16], F32, kind="ExternalInput")
    gmt = nc.dram_tensor("gmt", [16, 128], F32, kind="ExternalInput")
    out4 = nc.dram_tensor("out4", [BLOC, CH, 128, HW], F32, kind="ExternalOutput")

    with tile.TileContext(nc) as tc:
        from contextlib import ExitStack

        with ExitStack() as ctx:
            consts = ctx.enter_context(tc.tile_pool(name="consts", bufs=1))
            wstage = ctx.enter_context(tc.tile_pool(name="wstage", bufs=1))
            xresp = ctx.enter_context(tc.tile_pool(name="xresp", bufs=1))
            xpadp = ctx.enter_context(tc.tile_pool(name="xpadp", bufs=2))
            yp = ctx.enter_context(tc.tile_pool(name="yp", bufs=2))
            zp = ctx.enter_context(tc.tile_pool(name="zp", bufs=2))
            tfp = ctx.enter_context(tc.tile_pool(name="tfp", bufs=2))
            osp = ctx.enter_context(tc.tile_pool(name="osp", bufs=3))
            scrp = ctx.enter_context(tc.tile_pool(name="scrp", bufs=2))
            smallp = ctx.enter_context(tc.tile_pool(name="smallp", bufs=2))
            ps1 = ctx.enter_context(tc.tile_pool(name="ps1", bufs=2, space="PSUM"))
            ps2 = ctx.enter_context(tc.tile_pool(name="ps2", bufs=2, space="PSUM"))
            psdw = ctx.enter_context(tc.tile_pool(name="psdw", bufs=3, space="PSUM"))
            psm = ctx.enter_context(tc.tile_pool(name="psm", bufs=1, space="PSUM"))

            # ---- load constants ----
            aff_s = consts.tile([128, 4, 3 * C], F32)
            nc.sync.dma_start(out=aff_s[:], in_=aff[:])
            wt_s = consts.tile([128, BLOC, 4], F32)
            nc.sync.dma_start(out=wt_s[:], in_=wt[:])
            affb_s = consts.tile([128, 6], F32)
            nc.sync.dma_start(out=affb_s[:], in_=affb[:])
            dww_s = consts.tile([128, CH * 49], F32)
            nc.sync.dma_start(out=dww_s[:], in_=dww[:])
            dwb_s = consts.tile([128, CH], F32)
            nc.sync.dma_start(out=dwb_s[:], in_=dwb[:])
            ng_s = consts.tile([128, CH], F32)
            nc.sync.dma_start(out=ng_s[:], in_=ngt[:])
            nb_s = consts.tile([128, CH], F32)
            nc.sync.dma_start(out=nb_s[:], in_=nbt[:])
            p1b_s = consts.tile([128, 8], F32)
            nc.sync.dma_start(out=p1b_s[:], in_=p1b[:])
            p2b_s = consts.tile([128, CH], F32)
            nc.sync.dma_start(out=p2b_s[:], in_=p2b[:])
            gam_s = consts.tile([128, CH], F32)
            nc.sync.dma_start(out=gam_s[:], in_=gam[:])
            gmat_s = consts.tile([128, 16], F32)
            nc.sync.dma_start(out=gmat_s[:], in_=gmat[:])
            gmt_s = consts.tile([16, 128], F32)
            nc.sync.dma_start(out=gmt_s[:], in_=gmt[:])
            idm_s = consts.tile([128, 128], F32)
            nc.sync.dma_start(out=idm_s[:], in_=idm[:])

            # resident x (both samples, both chunks)
            x_s = {}
            for b in range(BLOC):
                for ch in range(CH):
                    t = xresp.tile([128, HW], F32, name=f"x{b}{ch}")
                    nc.sync.dma_start(out=t[:], in_=x4[b, ch])
                    x_s[(b, ch)] = t

            # staged fp32 weights -> fp8 (pre-scaled) + bf16 squares
            p1t_f = wstage.tile([128, CH, 4 * C], F32, tag="wstage")
            nc.sync.dma_start(out=p1t_f[:], in_=p1t[:])
            p1f8 = consts.tile([128, CH, 4 * C], FP8)
            nc.vector.tensor_scalar_mul(out=p1f8[:], in0=p1t_f[:], scalar1=SCL_P1)
            p1sq_b = consts.tile([128, CH, 4 * C], BF16)
            nc.scalar.square(out=p1sq_b[:], in_=p1t_f[:])

            p2t_f = wstage.tile([128, 8, C], F32, tag="wstage")
            nc.sync.dma_start(out=p2t_f[:], in_=p2t[:])
            p2f8 = consts.tile([128, 8, C], FP8)
            nc.vector.tensor_scalar_mul(out=p2f8[:], in0=p2t_f[:], scalar1=SCL_P2)

            gb_s = consts.tile([128, CH], F32)
            nc.vector.tensor_mul(out=gb_s[:], in0=gam_s[:], in1=p2b_s[:])
            gamsc = consts.tile([128, CH], F32)
            nc.vector.tensor_scalar_mul(out=gamsc[:], in0=gam_s[:],
                                        scalar1=float(1.0 / SCL_P2))
            dwbsq_s = consts.tile([128, CH], F32)
            nc.vector.tensor_mul(out=dwbsq_s[:], in0=dwb_s[:], in1=dwb_s[:])
            eps8 = consts.tile([128, 1], F32)
            nc.vector.memset(eps8[:], 1e-8)
            eps5 = consts.tile([128, 1], F32)
            nc.vector.memset(eps5[:], 1e-5)

            # scaled dw taps, then per-chunk diag stacks in PAIRS order
            dwsc = consts.tile([128, CH * 49], F32)
            nc.vector.tensor_scalar_mul(out=dwsc[:], in0=dww_s[:], scalar1=SCL_DW)
            dgp = consts.tile([128, CH, 2 * NPAIR, 128], FP8)
            for ch in range(CH):
                for i, (t0, t1) in enumerate(PAIRS):
                    nc.vector.tensor_scalar_mul(
                        out=dgp[:, ch, 2 * i, :], in0=idm_s[:],
                        scalar1=dwsc[:, ch * 49 + t0:ch * 49 + t0 + 1])
                    if t1 is None:
                        nc.vector.memset(dgp[:, ch, 2 * i + 1, :], 0.0)
                    else:
                        nc.vector.tensor_scalar_mul(
                            out=dgp[:, ch, 2 * i + 1, :], in0=idm_s[:],
                            scalar1=dwsc[:, ch * 49 + t1:ch * 49 + t1 + 1])

            # ---- engine sem pre-touches (single-sync-wait walrus) ----
            probe = consts.tile([128, 4], F32)
            for t_ in [dww_s, dwb_s, ng_s, nb_s, gam_s, p2b_s, affb_s, p1b_s,
                       idm_s, p1t_f, p2t_f]:
                sl1 = tuple([slice(0, 1)] + [0] * (len(t_[:].shape) - 2)
                            + [slice(0, 1)])
                nc.vector.tensor_copy(out=probe[0:1, 0:1], in_=t_[sl1])
            for t_ in [p1b_s, gam_s, dwb_s, p1t_f]:
                sl1 = tuple([slice(0, 1)] + [0] * (len(t_[:].shape) - 2)
                            + [slice(0, 1)])
                nc.scalar.copy(out=probe[0:1, 1:2], in_=t_[sl1])

            # ---- PE warmup touches: absorb one fresh semaphore each ----
            warm = psm.tile([2, 2], F32, tag="misc")
            touch = [aff_s, wt_s, gmat_s, gmt_s, p1sq_b, p1f8, p2f8, dgp]
            for tt_ in touch:
                sl2 = tuple([slice(0, 2)] + [0] * (len(tt_[:].shape) - 2)
                            + [slice(0, 2)])
                ap2 = tt_[sl2] if len(tt_[:].shape) > 2 else tt_[0:2, 0:2]
                nc.tensor.matmul(warm[:], ap2, ap2, start=True, stop=True)

            # ---- style affine for both samples: s = aff_w @ w_b + aff_b ----
            psty = psm.tile([128, 6, BLOC], F32, tag="misc")
            for m in range(6):
                for k in range(4):
                    nc.tensor.matmul(
                        psty[:, m, :],
                        aff_s[:, k, m * 128:(m + 1) * 128],
                        wt_s[:, :, k],
                        start=(k == 0),
                        stop=(k == 3),
                    )
            s_s = consts.tile([128, 6, BLOC], F32)
            for b in range(BLOC):
                nc.vector.tensor_add(out=s_s[:, :, b], in0=psty[:, :, b], in1=affb_s[:])
            # style = s1*s2 + s3 ; layout stl[:, ch*BLOC + b]
            stl = consts.tile([128, CH * BLOC], F32)
            tmp22 = consts.tile([128, CH, BLOC], F32)
            for b in range(BLOC):
                nc.vector.tensor_mul(
                    out=tmp22[:, :, b], in0=s_s[:, 0:2, b], in1=s_s[:, 2:4, b]
                )
                for ch in range(CH):
                    nc.vector.tensor_add(
                        out=stl[:, ch * BLOC + b:ch * BLOC + b + 1],
                        in0=tmp22[:, ch, b:b + 1],
                        in1=s_s[:, 4 + ch, b:b + 1],
                    )
            stlsq_b = consts.tile([128, CH * BLOC], BF16)
            nc.scalar.square(out=stlsq_b[:], in_=stl[:])

            # ---- dcoef: rsqrt(pw1_w^2 @ style^2 + 1e-8); then /SCL_P1 ----
            psd = psm.tile([128, 8, BLOC], F32, tag="misc")
            for o in range(8):
                for i in range(CH):
                    nc.tensor.matmul(
                        psd[:, o, :],
                        p1sq_b[:, i, o * 128:(o + 1) * 128],
                        stlsq_b[:, i * BLOC:(i + 1) * BLOC],
                        start=(i == 0),
                        stop=(i == CH - 1),
                    )
            dct = consts.tile([128, 8 * BLOC], F32)
            nc.scalar.activation(
                out=dct[:].rearrange("p (o b) -> p o b", b=BLOC),
                in_=psd[:],
                func=ACT.Sqrt,
                bias=eps8[:],
            )
            dcosc = consts.tile([128, 8 * BLOC], F32)
            nc.vector.reciprocal(out=dcosc[:], in_=dct[:])
            nc.vector.tensor_scalar_mul(out=dcosc[:], in0=dcosc[:],
                                        scalar1=float(1.0 / SCL_P1))

            y_tiles = {}
            sums_t = {}
            ysq_t = {}
            abf_t = {}

            def emit_dwconv(b, ch):
                if ch == 0:
                    y_tiles[b] = yp.tile([128, CH, HW], FP8, tag="y",
                                         name=f"y{b}")
                    sums_t[b] = smallp.tile([128, CH * NRB], F32, tag="sums",
                                            name=f"sums{b}")
                    ysq_t[b] = smallp.tile([128, CH * NRB], F32, tag="ysq",
                                           name=f"ysq{b}")
                y_s = y_tiles[b]
                sums = sums_t[b]
                ysqp = ysq_t[b]

                xpad = xpadp.tile([128, HP, WP], FP8, tag="xpad")
                # zero borders: top rows, bottom rows, left/right col strips
                nc.vector.memset(xpad[:, 0:3, :], 0.0)
                nc.vector.memset(xpad[:, 67:HP, :], 0.0)
                nc.vector.memset(xpad[:, 3:67, 0:3], 0.0)
                nc.vector.memset(xpad[:, 3:67, 67:WP], 0.0)
                xv = x_s[(b, ch)][:].rearrange("p (h w) -> p h w", w=64)
                nc.vector.tensor_copy(out=xpad[:, 3:67, 3:67], in_=xv)
                xflat = xpad[:].rearrange("p a b -> p (a b)")

                for grp in RBGROUPS:
                    ptiles = [psdw.tile([128, 504], F32, tag="dw",
                                        name=f"dw{b}{ch}")
                              for _ in grp]
                    for i, (t0, t1) in enumerate(PAIRS):
                        o0 = _toff(t0)
                        delta = (_toff(t1) - o0) if t1 is not None else 2
                        for gi, (r0, nr) in enumerate(grp):
                            fsz = nr * 72
                            base = r0 * 72 + o0
                            rhs = xflat[:, base:base + fsz].unsqueeze(1)
                            rhs.ap[1] = [delta, 2]
                            nc.tensor.matmul(
                                ptiles[gi][:, 0:fsz],
                                dgp[:, ch, 2 * i:2 * i + 2, :],
                                rhs,
                                start=(i == 0),
                                stop=(i == NPAIR - 1),
                                perf_mode=PM.DoubleRow,
                            )
                    for gi, (r0, nr) in enumerate(grp):
                        rbi = ROWBLKS.index((r0, nr))
                        fsz = nr * 72
                        pv = ptiles[gi][:, 0:fsz].rearrange(
                            "p (a c) -> p a c", c=72)[:, :, 3:67]
                        nc.scalar.activation(
                            out=y_s[:, ch, r0 * 64:(r0 + nr) * 64].rearrange(
                                "p (a c) -> p a c", c=64),
                            in_=pv,
                            func=ACT.Copy,
                            scale=float(1.0 / SCL_DW),
                            accum_out=sums[:, ch * NRB + rbi:ch * NRB + rbi + 1],
                        )
                        scr = scrp.tile([128, 7, 64], FP8, tag="scr")
                        nc.scalar.activation(
                            out=scr[:, 0:nr, :],
                            in_=pv,
                            func=ACT.Square,
                            scale=float(1.0 / SCL_DW),
                            accum_out=ysqp[:, ch * NRB + rbi:ch * NRB + rbi + 1],
                        )

            def emit_stats(b):
                y_s = y_tiles[b]
                sums = sums_t[b]
                ysqp = ysq_t[b]
                stats_c = smallp.tile([128, 4], F32, tag="stats")
                sc_sum = smallp.tile([128, CH], F32, tag="scs")
                nc.vector.tensor_reduce(
                    out=sc_sum[:],
                    in_=sums[:].rearrange("p (c k) -> p c k", k=NRB),
                    axis=mybir.AxisListType.X,
                    op=AOP.add,
                )
                ysq = smallp.tile([128, CH], F32, tag="ysqr")
                nc.vector.tensor_reduce(
                    out=ysq[:],
                    in_=ysqp[:].rearrange("p (c k) -> p c k", k=NRB),
                    axis=mybir.AxisListType.X,
                    op=AOP.add,
                )
                # adjust for dw bias: s' = s + 4096*b ; q' = q + 2*b*s + 4096*b^2
                nc.vector.scalar_tensor_tensor(
                    out=stats_c[:, 0:2],
                    in0=dwb_s[:],
                    scalar=float(HW),
                    in1=sc_sum[:],
                    op0=AOP.mult,
                    op1=AOP.add,
                )
                t_bs = smallp.tile([128, CH], F32, tag="tbs")
                nc.vector.tensor_mul(out=t_bs[:], in0=dwb_s[:], in1=sc_sum[:])
                t_q1 = smallp.tile([128, CH], F32, tag="tq1")
                nc.vector.scalar_tensor_tensor(
                    out=t_q1[:],
                    in0=t_bs[:],
                    scalar=2.0,
                    in1=ysq[:],
                    op0=AOP.mult,
                    op1=AOP.add,
                )
                nc.vector.scalar_tensor_tensor(
                    out=stats_c[:, 2:4],
                    in0=dwbsq_s[:],
                    scalar=float(HW),
                    in1=t_q1[:],
                    op0=AOP.mult,
                    op1=AOP.add,
                )
                gps = psm.tile([16, 4], F32, tag="misc")
                nc.tensor.matmul(gps[:], gmat_s[:], stats_c[:], start=True, stop=True)
                gsb = smallp.tile([16, 4], F32, tag="gsb")
                nc.vector.tensor_copy(out=gsb[:], in_=gps[:])
                grp4 = smallp.tile([16, 4], F32, tag="grp4")
                n_per_group = 8 * HW  # 32768
                nc.vector.tensor_scalar_mul(
                    out=grp4[:, 0:2], in0=gsb[:, 0:2], scalar1=1.0 / n_per_group
                )
                msq = smallp.tile([16, 2], F32, tag="msq")
                nc.vector.tensor_scalar_mul(
                    out=msq[:], in0=gsb[:, 2:4], scalar1=1.0 / n_per_group
                )
                mg2 = smallp.tile([16, 2], F32, tag="mg2")
                nc.vector.tensor_mul(out=mg2[:], in0=grp4[:, 0:2], in1=grp4[:, 0:2])
                var_t = smallp.tile([16, 2], F32, tag="var")
                nc.vector.tensor_sub(out=var_t[:], in0=msq[:], in1=mg2[:])
                sd_t = smallp.tile([16, 2], F32, tag="sd")
                nc.scalar.activation(out=sd_t[:], in_=var_t[:], func=ACT.Sqrt,
                                     bias=eps5[0:16, :])
                nc.vector.reciprocal(out=grp4[:, 2:4], in_=sd_t[:])
                bps = psm.tile([128, 4], F32, tag="misc")
                nc.tensor.matmul(bps[:], gmt_s[:], grp4[:], start=True, stop=True)
                mrc = smallp.tile([128, 4], F32, tag="mrc")
                nc.vector.tensor_copy(out=mrc[:], in_=bps[:])

                # per-channel affine A*y + B folding groupnorm affine, style, dw bias
                abf = smallp.tile([128, 4], F32, tag="abf", name=f"abf{b}")
                a0t = smallp.tile([128, 2], F32, tag="a0t")
                for ch in range(CH):
                    stl_c = stl[:, ch * BLOC + b:ch * BLOC + b + 1]
                    nc.vector.tensor_mul(
                        out=a0t[:, ch:ch + 1],
                        in0=ng_s[:, ch:ch + 1],
                        in1=mrc[:, 2 + ch:3 + ch],
                    )
                    nc.vector.tensor_mul(
                        out=abf[:, ch:ch + 1], in0=a0t[:, ch:ch + 1], in1=stl_c
                    )
                    t2 = smallp.tile([128, 1], F32, tag="t2")
                    nc.vector.tensor_mul(
                        out=t2[:], in0=mrc[:, ch:ch + 1], in1=a0t[:, ch:ch + 1]
                    )
                    t3 = smallp.tile([128, 1], F32, tag="t3")
                    nc.vector.tensor_sub(out=t3[:], in0=nb_s[:, ch:ch + 1], in1=t2[:])
                    t4 = smallp.tile([128, 1], F32, tag="t4")
                    nc.vector.tensor_mul(out=t4[:], in0=t3[:], in1=stl_c)
                    nc.vector.scalar_tensor_tensor(
                        out=abf[:, 2 + ch:3 + ch],
                        in0=abf[:, ch:ch + 1],
                        scalar=dwb_s[:, ch:ch + 1],
                        in1=t4[:],
                        op0=AOP.mult,
                        op1=AOP.add,
                    )
                abf_t[b] = abf
                # modulation: y <- A*y + B in place (DVE, per chunk)
                for ch in range(CH):
                    nc.vector.tensor_scalar(
                        out=y_s[:, ch, :],
                        in0=y_s[:, ch, :],
                        scalar1=abf[:, ch:ch + 1],
                        scalar2=abf[:, 2 + ch:3 + ch],
                        op0=AOP.mult,
                        op1=AOP.add,
                    )

            def emit_pwblk(b, blk):
                y_s = y_tiles[b]
                sl = slice(blk * BLKN, (blk + 1) * BLKN)
                zg = zp.tile([128, 8, BLKN], FP8, tag="zg", name=f"zg{b}{blk}")
                for o in range(8):
                    pz = ps1.tile([128, BLKN], F32, tag="pz")
                    nc.tensor.matmul(
                        pz[:],
                        p1f8[:, :, o * 128:(o + 1) * 128],
                        y_s[:, :, sl],
                        start=True,
                        stop=True,
                        perf_mode=PM.DoubleRow,
                    )
                    nc.scalar.activation(
                        out=zg[:, o, :],
                        in_=pz[:],
                        func=ACT.Gelu,
                        bias=p1b_s[:, o:o + 1],
                        scale=dcosc[:, o * BLOC + b:o * BLOC + b + 1],
                    )
                for c in range(CH):
                    p2ps = ps2.tile([128, BLKN], F32, tag="p2")
                    for q in range(4):
                        nc.tensor.matmul(
                            p2ps[:],
                            p2f8[:, 2 * q:2 * q + 2, c * 128:(c + 1) * 128],
                            zg[:, 2 * q:2 * q + 2, :],
                            start=(q == 0),
                            stop=(q == 3),
                            perf_mode=PM.DoubleRow,
                        )
                    tf = tfp.tile([128, BLKN], F32, tag="tf")
                    nc.scalar.activation(
                        out=tf[:],
                        in_=p2ps[:],
                        func=ACT.Identity,
                        bias=gb_s[:, c:c + 1],
                        scale=gamsc[:, c:c + 1],
                    )
                    ost = osp.tile([128, BLKN], F32, tag="os")
                    nc.vector.tensor_add(out=ost[:], in0=tf[:],
                                         in1=x_s[(b, c)][:, sl])
                    nc.sync.dma_start(out=out4[b, c, :, sl], in_=ost[:])

            # ---- main schedule: dw(0) | dw(1) interleaved with pw(0) | pw(1)
            emit_dwconv(0, 0)
            emit_dwconv(0, 1)
            emit_stats(0)
            emit_dwconv(1, 0)
            for blk in range(4):
                emit_pwblk(0, blk)
            emit_dwconv(1, 1)
            for blk in range(4, NBLK):
                emit_pwblk(0, blk)
            emit_stats(1)
            for blk in range(NBLK):
                emit_pwblk(1, blk)

    return nc


_NC = None


def _get_nc():
    global _NC
    if _NC is None:
        _NC = build_nc()
    return _NC


def _prep_maps(x, w, aff_w, aff_b, dw_w, dw_b, norm_g, norm_b, pw1_w, pw1_b, pw2_w,
               pw2_b, gamma):
    f = np.float32
    ct = lambda a: np.ascontiguousarray(a, dtype=f)
    common = {
        "aff": ct(aff_w.T.reshape(4, 128, 3 * C).transpose(1, 0, 2)),
        "affb": ct(aff_b.reshape(6, 128).T),
        "dww": ct(dw_w.reshape(C, 49).reshape(CH, 128, 49).transpose(1, 0, 2)
                  .reshape(128, CH * 49)),
        "dwb": ct(dw_b.reshape(CH, 128).T),
        "ngt": ct(norm_g.reshape(CH, 128).T),
        "nbt": ct(norm_b.reshape(CH, 128).T),
        "p1t": ct(pw1_w.T.reshape(CH, 128, 4 * C).transpose(1, 0, 2)),
        "p1b": ct(pw1_b.reshape(8, 128).T),
        "p2t": ct(pw2_w.T.reshape(8, 128, C).transpose(1, 0, 2)),
        "p2b": ct(pw2_b.reshape(CH, 128).T),
        "gam": ct(gamma.reshape(CH, 128).T),
        "idm": np.eye(128, dtype=f),
        "gmat": ct((np.arange(128)[:, None] // 8 == np.arange(16)[None, :])),
        "gmt": ct((np.arange(16)[:, None] == np.arange(128)[None, :] // 8)),
    }
    in_maps = []
    for i in range(NCORES):
        sl = slice(i * BLOC, (i + 1) * BLOC)
        m = dict(common)
        m["x4"] = ct(x[sl].reshape(BLOC, CH, 128, HW))
        m["wt"] = ct(w[sl].reshape(BLOC, 4, 128).transpose(2, 0, 1))
        in_maps.append(m)
    return in_maps


LAST_EXEC_NS = None


def _run(inputs, trace=False):
    global LAST_EXEC_NS
    nc = _get_nc()
    in_maps = _prep_maps(**inputs)
    res = run_bass_kernel_spmd(nc, in_maps, core_ids=list(range(NCORES)), trace=trace)
    LAST_EXEC_NS = res.exec_time_ns
    outs = [res.results[i]["out4"].reshape(BLOC, C, H, W) for i in range(NCORES)]
    return np.concatenate(outs, axis=0).astype(np.float32)


def kernel(**inputs):
    return _run({k: np.asarray(v) for k, v in inputs.items()}, trace=False)


# revision 5
# speedup vs baseline: 2.5159x; 1.0945x over previous
"""ConvNeXtSynthesisLayer Trainium2 kernel (8 NeuronCores, data-parallel over batch).

Self-contained: hardcodes shapes B=16, C=256, H=W=64, WD=512, K=7.
Each core processes 2 samples end-to-end on-chip. v3: the depthwise 7x7 conv and
both pointwise convs run on the PE in fp8 DoubleRow mode (K=256 per matmul; the
dwconv pairs two taps per matmul via a hand-built 2-k-tile access pattern with
even byte deltas). All fp8/bf16 weight tensors (incl. the 49 diagonal tap
matrices) are precomputed on the host and DMAed. GroupNorm stats come from the
DVE PSUM drains (tensor_scalar with accum_out) plus one fused y^2 pass; the
per-channel affine (GroupNorm x style modulation) is one DVE tensor_scalar pass;
demodulation and the fp8 pre-scales fold into drain scales.
"""

import os
import sys

sys.path.insert(0, "/opt/trn_rl_repo")

import numpy as np
import ml_dtypes

import concourse.bass as bass
import concourse.tile as tile
from concourse import mybir
from concourse.bass_utils import run_bass_kernel_spmd


def _spill_multiwaits(ordered):
    """This walrus build accepts a single sync wait per instruction; move each
    extra wait onto an injected same-engine NoOp placed just before it."""
    for bb, insts in list(ordered.items()):
        out = []
        for inst in insts:
            si = getattr(inst, "sync_info", None)
            eng = getattr(inst, "engine", None)
            if si is not None and eng is not None and len(si.on_wait) > 1:
                waits = list(si.on_wait)
                for j, w in enumerate(waits[:-1]):
                    out.append(
                        mybir.InstNoOp(
                            name=f"{inst.name}-ws{j}",
                            engine=eng,
                            sync_info=mybir.SyncInfo(on_wait=[w], on_update=[]),
                            ins=[],
                            outs=[],
                        )
                    )
                inst.sync_info = mybir.SyncInfo(
                    on_wait=[waits[-1]], on_update=list(si.on_update)
                )
            out.append(inst)
        insts[:] = out


_OrigTCW = tile.TileClockWait


class _SpillTCW:
    def __init__(self, tc, ordered):
        self._inner = _OrigTCW(tc, ordered)
        self._tc = tc
        self._ordered = ordered

    def assign_waits(self, *a, **k):
        r = self._inner.assign_waits(*a, **k)
        _spill_multiwaits(self._ordered)
        return r

    def add_sem_waits(self, raw_inst, *a, **k):
        # kernel-tail drain: split a multi-wait drain into single-wait drains
        # (order-insensitive — the all-engine barrier follows them all)
        r = self._inner.add_sem_waits(raw_inst, *a, **k)
        si = getattr(raw_inst, "sync_info", None)
        if si is not None and len(si.on_wait) > 1:
            waits = list(si.on_wait)
            raw_inst.sync_info = mybir.SyncInfo(
                on_wait=waits[:1], on_update=list(si.on_update)
            )
            for w in waits[1:]:
                d = self._tc.nc.sync.drain()
                d.ins.sync_info = mybir.SyncInfo(on_wait=[w], on_update=[])
        return r

    def __getattr__(self, k):
        return getattr(self._inner, k)


tile.TileClockWait = _SpillTCW

F32 = mybir.dt.float32
BF16 = mybir.dt.bfloat16
FP8 = mybir.dt.float8e4
AOP = mybir.AluOpType
ACT = mybir.ActivationFunctionType
PM = mybir.MatmulPerfMode

B, C, H, W = 16, 256, 64, 64
WD, K7 = 512, 7
NCORES = 8
BLOC = B // NCORES          # samples per core = 2
CH = C // 128               # channel chunks = 2
HW = H * W                  # 4096
NBLK = 8                    # pwconv pixel blocks of 512
BLKN = HW // NBLK           # 512
HP, WP = 71, 72             # padded image (3 top, 4 bottom junk; data at col 3+j)

SCL_DW = 64.0               # fp8 pre-scale of depthwise taps
SCL_P1 = 32.0               # fp8 pre-scale of pwconv1 weights
SCL_P2 = 32.0               # fp8 pre-scale of pwconv2 weights

# tap pairs with EVEN flat-offset deltas (DoubleRow k-tile stride must be
# 2-byte aligned); offsets are dy*72+dx in the padded image
PAIRS = []
for dy in range(7):
    PAIRS.append((dy * 7 + 0, dy * 7 + 2))
    PAIRS.append((dy * 7 + 4, dy * 7 + 6))
    PAIRS.append((dy * 7 + 1, dy * 7 + 3))
for dy in range(0, 6, 2):
    PAIRS.append((dy * 7 + 5, (dy + 1) * 7 + 5))
PAIRS.append((6 * 7 + 5, None))
assert len(PAIRS) == 25
NPAIR = 25


def _toff(t):
    return (t // 7) * 72 + (t % 7)


# 7-row output blocks in padded geometry (last block 1 row)
ROWBLKS = [(r, min(7, 64 - r)) for r in range(0, 64, 7)]
NRB = len(ROWBLKS)          # 10
# psum-bank groups of 2, double-buffered across 4 banks
RBGROUPS = [ROWBLKS[i:i + 2] for i in range(0, NRB, 2)]


def build_nc():
    nc = bass.Bass()

    # ---- DRAM I/O (per-core shards; weights replicated) ----
    x4 = nc.dram_tensor("x4", [BLOC, CH, 128, HW], F32, kind="ExternalInput")
    wt = nc.dram_tensor("wt", [128, BLOC, 4], F32, kind="ExternalInput")
    aff = nc.dram_tensor("aff", [128, 4, 3 * C], F32, kind="ExternalInput")
    affb = nc.dram_tensor("affb", [128, 6], F32, kind="ExternalInput")
    dwb = nc.dram_tensor("dwb", [128, CH], F32, kind="ExternalInput")
    ngt = nc.dram_tensor("ngt", [128, CH], F32, kind="ExternalInput")
    nbt = nc.dram_tensor("nbt", [128, CH], F32, kind="ExternalInput")
    p1f = nc.dram_tensor("p1f", [128, CH, 4 * C], FP8, kind="ExternalInput")
    p1sq = nc.dram_tensor("p1sq", [128, CH, 4 * C], BF16, kind="ExternalInput")
    p1b = nc.dram_tensor("p1b", [128, 8], F32, kind="ExternalInput")
    p2f = nc.dram_tensor("p2f", [128, 8, C], FP8, kind="ExternalInput")
    p2b = nc.dram_tensor("p2b", [128, CH], F32, kind="ExternalInput")
    gam = nc.dram_tensor("gam", [128, CH], F32, kind="ExternalInput")
    dgpd = nc.dram_tensor("dgpd", [128, CH, 2 * NPAIR, 128], FP8,
                          kind="ExternalInput")
    gmat = nc.dram_tensor("gmat", [128, 16], F32, kind="ExternalInput")
    gmt = nc.dram_tensor("gmt", [16, 128], F32, kind="ExternalInput")
    out4 = nc.dram_tensor("out4", [BLOC, CH, 128, HW], F32, kind="ExternalOutput")

    with tile.TileContext(nc) as tc:
        from contextlib import ExitStack

        with ExitStack() as ctx:
            consts = ctx.enter_context(tc.tile_pool(name="consts", bufs=1))
            xresp = ctx.enter_context(tc.tile_pool(name="xresp", bufs=1))
            xpadp = ctx.enter_context(tc.tile_pool(name="xpadp", bufs=2))
            yp = ctx.enter_context(tc.tile_pool(name="yp", bufs=2))
            zp = ctx.enter_context(tc.tile_pool(name="zp", bufs=2))
            tfp = ctx.enter_context(tc.tile_pool(name="tfp", bufs=2))
            osp = ctx.enter_context(tc.tile_pool(name="osp", bufs=3))
            scrp = ctx.enter_context(tc.tile_pool(name="scrp", bufs=2))
            smallp = ctx.enter_context(tc.tile_pool(name="smallp", bufs=2))
            psdw = ctx.enter_context(tc.tile_pool(name="psdw", bufs=4, space="PSUM"))
            pspz = ctx.enter_context(tc.tile_pool(name="pspz", bufs=2, space="PSUM"))
            psp2 = ctx.enter_context(tc.tile_pool(name="psp2", bufs=1, space="PSUM"))
            psmc = ctx.enter_context(tc.tile_pool(name="psmc", bufs=1, space="PSUM"))

            # ---- resident x (both samples, both chunks) ----
            x_s = {}
            for b in range(BLOC):
                for ch in range(CH):
                    t = xresp.tile([128, HW], F32, name=f"x{b}{ch}")
                    nc.sync.dma_start(out=t[:], in_=x4[b, ch])
                    x_s[(b, ch)] = t

            # ---- load constants ----
            dgp = consts.tile([128, CH, 2 * NPAIR, 128], FP8)
            nc.sync.dma_start(out=dgp[:], in_=dgpd[:])
            aff_s = consts.tile([128, 4, 3 * C], F32)
            nc.sync.dma_start(out=aff_s[:], in_=aff[:])
            wt_s = consts.tile([128, BLOC, 4], F32)
            nc.sync.dma_start(out=wt_s[:], in_=wt[:])
            affb_s = consts.tile([128, 6], F32)
            nc.sync.dma_start(out=affb_s[:], in_=affb[:])
            dwb_s = consts.tile([128, CH], F32)
            nc.sync.dma_start(out=dwb_s[:], in_=dwb[:])
            ng_s = consts.tile([128, CH], F32)
            nc.sync.dma_start(out=ng_s[:], in_=ngt[:])
            nb_s = consts.tile([128, CH], F32)
            nc.sync.dma_start(out=nb_s[:], in_=nbt[:])
            p1b_s = consts.tile([128, 8], F32)
            nc.sync.dma_start(out=p1b_s[:], in_=p1b[:])
            p2b_s = consts.tile([128, CH], F32)
            nc.sync.dma_start(out=p2b_s[:], in_=p2b[:])
            gam_s = consts.tile([128, CH], F32)
            nc.sync.dma_start(out=gam_s[:], in_=gam[:])
            gmat_s = consts.tile([128, 16], F32)
            nc.sync.dma_start(out=gmat_s[:], in_=gmat[:])
            gmt_s = consts.tile([16, 128], F32)
            nc.sync.dma_start(out=gmt_s[:], in_=gmt[:])
            p1f8 = consts.tile([128, CH, 4 * C], FP8)
            nc.sync.dma_start(out=p1f8[:], in_=p1f[:])
            p1sq_b = consts.tile([128, CH, 4 * C], BF16)
            nc.sync.dma_start(out=p1sq_b[:], in_=p1sq[:])
            p2f8 = consts.tile([128, 8, C], FP8)
            nc.sync.dma_start(out=p2f8[:], in_=p2f[:])

            # ---- engine sem pre-touches (single-sync-wait walrus) ----
            probe = consts.tile([128, 4], F32)
            for t_ in [dwb_s, ng_s, nb_s, gam_s, p2b_s, affb_s, p1b_s]:
                nc.vector.tensor_copy(out=probe[0:1, 0:1], in_=t_[0:1, 0:1])
            for t_ in [p1b_s, gam_s]:
                nc.scalar.copy(out=probe[0:1, 1:2], in_=t_[0:1, 0:1])

            # ---- PE warmup touches: absorb one fresh semaphore each ----
            warm = psmc.tile([2, 2], F32, tag="misc")
            touch = [aff_s, wt_s, gmat_s, gmt_s, p1sq_b, p1f8, p2f8, dgp]
            for tt_ in touch:
                sl2 = tuple([slice(0, 2)] + [0] * (len(tt_[:].shape) - 2)
                            + [slice(0, 2)])
                ap2 = tt_[sl2] if len(tt_[:].shape) > 2 else tt_[0:2, 0:2]
                nc.tensor.matmul(warm[:], ap2, ap2, start=True, stop=True)

            gb_s = consts.tile([128, CH], F32)
            nc.vector.tensor_mul(out=gb_s[:], in0=gam_s[:], in1=p2b_s[:])
            gamsc = consts.tile([128, CH], F32)
            nc.vector.tensor_scalar_mul(out=gamsc[:], in0=gam_s[:],
                                        scalar1=float(1.0 / SCL_P2))
            dwbsq_s = consts.tile([128, CH], F32)
            nc.vector.tensor_mul(out=dwbsq_s[:], in0=dwb_s[:], in1=dwb_s[:])
            eps8 = consts.tile([128, 1], F32)
            nc.vector.memset(eps8[:], 1e-8)
            eps5 = consts.tile([128, 1], F32)
            nc.vector.memset(eps5[:], 1e-5)

            # ---- style affine for both samples: s = aff_w @ w_b + aff_b ----
            psty = psmc.tile([128, 6, BLOC], F32, tag="misc")
            for m in range(6):
                for k in range(4):
                    nc.tensor.matmul(
                        psty[:, m, :],
                        aff_s[:, k, m * 128:(m + 1) * 128],
                        wt_s[:, :, k],
                        start=(k == 0),
                        stop=(k == 3),
                    )
            s_s = consts.tile([128, 6, BLOC], F32)
            for b in range(BLOC):
                nc.vector.tensor_add(out=s_s[:, :, b], in0=psty[:, :, b], in1=affb_s[:])
            # style = s1*s2 + s3 ; layout stl[:, ch*BLOC + b]
            stl = consts.tile([128, CH * BLOC], F32)
            tmp22 = consts.tile([128, CH, BLOC], F32)
            for b in range(BLOC):
                nc.vector.tensor_mul(
                    out=tmp22[:, :, b], in0=s_s[:, 0:2, b], in1=s_s[:, 2:4, b]
                )
                for ch in range(CH):
                    nc.vector.tensor_add(
                        out=stl[:, ch * BLOC + b:ch * BLOC + b + 1],
                        in0=tmp22[:, ch, b:b + 1],
                        in1=s_s[:, 4 + ch, b:b + 1],
                    )
            stlsq_b = consts.tile([128, CH * BLOC], BF16)
            nc.scalar.square(out=stlsq_b[:], in_=stl[:])

            # ---- dcoef: rsqrt(pw1_w^2 @ style^2 + 1e-8); then /SCL_P1 ----
            psd = psmc.tile([128, 8, BLOC], F32, tag="misc")
            for o in range(8):
                for i in range(CH):
                    nc.tensor.matmul(
                        psd[:, o, :],
                        p1sq_b[:, i, o * 128:(o + 1) * 128],
                        stlsq_b[:, i * BLOC:(i + 1) * BLOC],
                        start=(i == 0),
                        stop=(i == CH - 1),
                    )
            dct = consts.tile([128, 8 * BLOC], F32)
            nc.scalar.activation(
                out=dct[:].rearrange("p (o b) -> p o b", b=BLOC),
                in_=psd[:],
                func=ACT.Sqrt,
                bias=eps8[:],
            )
            dcosc = consts.tile([128, 8 * BLOC], F32)
            nc.vector.reciprocal(out=dcosc[:], in_=dct[:])
            nc.vector.tensor_scalar_mul(out=dcosc[:], in0=dcosc[:],
                                        scalar1=float(1.0 / SCL_P1))

            y_tiles = {}
            sums_t = {}
            ysq_t = {}

            def emit_dwconv(b, ch):
                if ch == 0:
                    y_tiles[b] = yp.tile([128, CH, HW], FP8, tag="y",
                                         name=f"y{b}")
                    sums_t[b] = smallp.tile([128, CH * NRB], F32, tag="sums",
                                            name=f"sums{b}")
                    ysq_t[b] = smallp.tile([128, CH], F32, tag="ysq",
                                           name=f"ysq{b}")
                y_s = y_tiles[b]
                sums = sums_t[b]

                xpad = xpadp.tile([128, HP, WP], FP8, tag="xpad")
                # zero borders: top rows, bottom rows, left/right col strips
                nc.vector.memset(xpad[:, 0:3, :], 0.0)
                nc.vector.memset(xpad[:, 67:HP, :], 0.0)
                nc.vector.memset(xpad[:, 3:67, 0:3], 0.0)
                nc.vector.memset(xpad[:, 3:67, 67:WP], 0.0)
                xv = x_s[(b, ch)][:].rearrange("p (h w) -> p h w", w=64)
                nc.vector.tensor_copy(out=xpad[:, 3:67, 3:67], in_=xv)
                xflat = xpad[:].rearrange("p a b -> p (a b)")

                for grp in RBGROUPS:
                    ptiles = [psdw.tile([128, 504], F32, tag="dw",
                                        name=f"dw{b}{ch}")
                              for _ in grp]
                    for i, (t0, t1) in enumerate(PAIRS):
                        o0 = _toff(t0)
                        delta = (_toff(t1) - o0) if t1 is not None else 2
                        for gi, (r0, nr) in enumerate(grp):
                            fsz = nr * 72
                            base = r0 * 72 + o0
                            rhs = xflat[:, base:base + fsz].unsqueeze(1)
                            rhs.ap[1] = [delta, 2]
                            nc.tensor.matmul(
                                ptiles[gi][:, 0:fsz],
                                dgp[:, ch, 2 * i:2 * i + 2, :],
                                rhs,
                                start=(i == 0),
                                stop=(i == NPAIR - 1),
                                perf_mode=PM.DoubleRow,
                            )
                    for gi, (r0, nr) in enumerate(grp):
                        rbi = ROWBLKS.index((r0, nr))
                        fsz = nr * 72
                        pv = ptiles[gi][:, 0:fsz].rearrange(
                            "p (a c) -> p a c", c=72)[:, :, 3:67]
                        # DVE drain: y = psum/SCL (fp8), accum -> channel sums
                        nc.vector.tensor_scalar(
                            out=y_s[:, ch, r0 * 64:(r0 + nr) * 64].rearrange(
                                "p (a c) -> p a c", c=64),
                            in0=pv,
                            scalar1=float(1.0 / SCL_DW),
                            scalar2=0.0,
                            op0=AOP.mult,
                            op1=AOP.add,
                            accum_out=sums[:, ch * NRB + rbi:ch * NRB + rbi + 1],
                        )
                # fused y^2 sum for this chunk (single DVE pass over y)
                scr = scrp.tile([128, HW], FP8, tag="scr")
                nc.vector.scalar_tensor_tensor(
                    out=scr[:],
                    in0=y_s[:, ch, :],
                    scalar=1.0,
                    in1=y_s[:, ch, :],
                    op0=AOP.mult,
                    op1=AOP.mult,
                    accum_out=ysq_t[b][:, ch:ch + 1],
                )

            def emit_stats(b):
                y_s = y_tiles[b]
                sums = sums_t[b]
                ysq = ysq_t[b]
                stats_c = smallp.tile([128, 4], F32, tag="stats")
                sc_sum = smallp.tile([128, CH], F32, tag="scs")
                nc.vector.tensor_reduce(
                    out=sc_sum[:],
                    in_=sums[:].rearrange("p (c k) -> p c k", k=NRB),
                    axis=mybir.AxisListType.X,
                    op=AOP.add,
                )
                # adjust for dw bias: s' = s + 4096*b ; q' = q + 2*b*s + 4096*b^2
                nc.vector.scalar_tensor_tensor(
                    out=stats_c[:, 0:2],
                    in0=dwb_s[:],
                    scalar=float(HW),
                    in1=sc_sum[:],
                    op0=AOP.mult,
                    op1=AOP.add,
                )
                t_bs = smallp.tile([128, CH], F32, tag="tbs")
                nc.vector.tensor_mul(out=t_bs[:], in0=dwb_s[:], in1=sc_sum[:])
                t_q1 = smallp.tile([128, CH], F32, tag="tq1")
                nc.vector.scalar_tensor_tensor(
                    out=t_q1[:],
                    in0=t_bs[:],
                    scalar=2.0,
                    in1=ysq[:],
                    op0=AOP.mult,
                    op1=AOP.add,
                )
                nc.vector.scalar_tensor_tensor(
                    out=stats_c[:, 2:4],
                    in0=dwbsq_s[:],
                    scalar=float(HW),
                    in1=t_q1[:],
                    op0=AOP.mult,
                    op1=AOP.add,
                )
                gps = psmc.tile([16, 4], F32, tag="misc")
                nc.tensor.matmul(gps[:], gmat_s[:], stats_c[:], start=True, stop=True)
                gsb = smallp.tile([16, 4], F32, tag="gsb")
                nc.vector.tensor_copy(out=gsb[:], in_=gps[:])
                grp4 = smallp.tile([16, 4], F32, tag="grp4")
                n_per_group = 8 * HW  # 32768
                nc.vector.tensor_scalar_mul(
                    out=grp4[:, 0:2], in0=gsb[:, 0:2], scalar1=1.0 / n_per_group
                )
                msq = smallp.tile([16, 2], F32, tag="msq")
                nc.vector.tensor_scalar_mul(
                    out=msq[:], in0=gsb[:, 2:4], scalar1=1.0 / n_per_group
                )
                mg2 = smallp.tile([16, 2], F32, tag="mg2")
                nc.vector.tensor_mul(out=mg2[:], in0=grp4[:, 0:2], in1=grp4[:, 0:2])
                var_t = smallp.tile([16, 2], F32, tag="var")
                nc.vector.tensor_sub(out=var_t[:], in0=msq[:], in1=mg2[:])
                sd_t = smallp.tile([16, 2], F32, tag="sd")
                nc.scalar.activation(out=sd_t[:], in_=var_t[:], func=ACT.Sqrt,
                                     bias=eps5[0:16, :])
                nc.vector.reciprocal(out=grp4[:, 2:4], in_=sd_t[:])
                bps = psmc.tile([128, 4], F32, tag="misc")
                nc.tensor.matmul(bps[:], gmt_s[:], grp4[:], start=True, stop=True)
                mrc = smallp.tile([128, 4], F32, tag="mrc")
                nc.vector.tensor_copy(out=mrc[:], in_=bps[:])

                # per-channel affine A*y + B folding groupnorm affine, style, dw bias
                abf = smallp.tile([128, 4], F32, tag="abf", name=f"abf{b}")
                a0t = smallp.tile([128, 2], F32, tag="a0t")
                for ch in range(CH):
                    stl_c = stl[:, ch * BLOC + b:ch * BLOC + b + 1]
                    nc.vector.tensor_mul(
                        out=a0t[:, ch:ch + 1],
                        in0=ng_s[:, ch:ch + 1],
                        in1=mrc[:, 2 + ch:3 + ch],
                    )
                    nc.vector.tensor_mul(
                        out=abf[:, ch:ch + 1], in0=a0t[:, ch:ch + 1], in1=stl_c
                    )
                    t2 = smallp.tile([128, 1], F32, tag="t2")
                    nc.vector.tensor_mul(
                        out=t2[:], in0=mrc[:, ch:ch + 1], in1=a0t[:, ch:ch + 1]
                    )
                    t3 = smallp.tile([128, 1], F32, tag="t3")
                    nc.vector.tensor_sub(out=t3[:], in0=nb_s[:, ch:ch + 1], in1=t2[:])
                    t4 = smallp.tile([128, 1], F32, tag="t4")
                    nc.vector.tensor_mul(out=t4[:], in0=t3[:], in1=stl_c)
                    nc.vector.scalar_tensor_tensor(
                        out=abf[:, 2 + ch:3 + ch],
                        in0=abf[:, ch:ch + 1],
                        scalar=dwb_s[:, ch:ch + 1],
                        in1=t4[:],
                        op0=AOP.mult,
                        op1=AOP.add,
                    )
                # modulation: y <- A*y + B in place (DVE, per chunk)
                for ch in range(CH):
                    nc.vector.tensor_scalar(
                        out=y_s[:, ch, :],
                        in0=y_s[:, ch, :],
                        scalar1=abf[:, ch:ch + 1],
                        scalar2=abf[:, 2 + ch:3 + ch],
                        op0=AOP.mult,
                        op1=AOP.add,
                    )

            def emit_pwblk(b, blk):
                y_s = y_tiles[b]
                sl = slice(blk * BLKN, (blk + 1) * BLKN)
                zg = zp.tile([128, 8, BLKN], FP8, tag="zg", name=f"zg{b}{blk}")
                for o in range(8):
                    pz = pspz.tile([128, BLKN], F32, tag="pz")
                    nc.tensor.matmul(
                        pz[:],
                        p1f8[:, :, o * 128:(o + 1) * 128],
                        y_s[:, :, sl],
                        start=True,
                        stop=True,
                        perf_mode=PM.DoubleRow,
                    )
                    nc.scalar.activation(
                        out=zg[:, o, :],
                        in_=pz[:],
                        func=ACT.Gelu,
                        bias=p1b_s[:, o:o + 1],
                        scale=dcosc[:, o * BLOC + b:o * BLOC + b + 1],
                    )
                for c in range(CH):
                    p2ps = psp2.tile([128, BLKN], F32, tag="p2")
                    for q in range(4):
                        nc.tensor.matmul(
                            p2ps[:],
                            p2f8[:, 2 * q:2 * q + 2, c * 128:(c + 1) * 128],
                            zg[:, 2 * q:2 * q + 2, :],
                            start=(q == 0),
                            stop=(q == 3),
                            perf_mode=PM.DoubleRow,
                        )
                    tf = tfp.tile([128, BLKN], F32, tag="tf")
                    nc.scalar.activation(
                        out=tf[:],
                        in_=p2ps[:],
                        func=ACT.Identity,
                        bias=gb_s[:, c:c + 1],
                        scale=gamsc[:, c:c + 1],
                    )
                    ost = osp.tile([128, BLKN], F32, tag="os")
                    nc.vector.tensor_add(out=ost[:], in0=tf[:],
                                         in1=x_s[(b, c)][:, sl])
                    nc.sync.dma_start(out=out4[b, c, :, sl], in_=ost[:])

            # ---- main schedule ----
            emit_dwconv(0, 0)
            emit_dwconv(0, 1)
            emit_dwconv(1, 0)
            emit_stats(0)
            for blk in range(4):
                emit_pwblk(0, blk)
            emit_dwconv(1, 1)
            for blk in range(4, NBLK):
                emit_pwblk(0, blk)
            emit_stats(1)
            for blk in range(NBLK):
                emit_pwblk(1, blk)

    return nc


_NC = None


def _get_nc():
    global _NC
    if _NC is None:
        _NC = build_nc()
    return _NC


def _prep_maps(x, w, aff_w, aff_b, dw_w, dw_b, norm_g, norm_b, pw1_w, pw1_b, pw2_w,
               pw2_b, gamma):
    f = np.float32
    ct = lambda a: np.ascontiguousarray(a, dtype=f)
    FP8NP = ml_dtypes.float8_e4m3
    BF16NP = ml_dtypes.bfloat16

    p1t = pw1_w.T.reshape(CH, 128, 4 * C).transpose(1, 0, 2).astype(f)
    p2t = pw2_w.T.reshape(8, 128, C).transpose(1, 0, 2).astype(f)
    dww = (dw_w.reshape(C, 49).reshape(CH, 128, 49).transpose(1, 0, 2)
           .astype(f))  # [128, CH, 49]

    # diag tap stacks in PAIRS order: dgp[p, ch, 2i+j, :] = diag(w_tap * SCL)
    dgp = np.zeros((128, CH, 2 * NPAIR, 128), dtype=f)
    eye = np.eye(128, dtype=f)
    for ch in range(CH):
        for i, (t0, t1) in enumerate(PAIRS):
            dgp[:, ch, 2 * i, :] = eye * (dww[:, ch, t0] * SCL_DW)[:, None]
            if t1 is not None:
                dgp[:, ch, 2 * i + 1, :] = eye * (dww[:, ch, t1] * SCL_DW)[:, None]

    common = {
        "aff": ct(aff_w.T.reshape(4, 128, 3 * C).transpose(1, 0, 2)),
        "affb": ct(aff_b.reshape(6, 128).T),
        "dwb": ct(dw_b.reshape(CH, 128).T),
        "ngt": ct(norm_g.reshape(CH, 128).T),
        "nbt": ct(norm_b.reshape(CH, 128).T),
        "p1f": np.ascontiguousarray((p1t * SCL_P1).astype(FP8NP)),
        "p1sq": np.ascontiguousarray((p1t * p1t).astype(BF16NP)),
        "p1b": ct(pw1_b.reshape(8, 128).T),
        "p2f": np.ascontiguousarray((p2t * SCL_P2).astype(FP8NP)),
        "p2b": ct(pw2_b.reshape(CH, 128).T),
        "gam": ct(gamma.reshape(CH, 128).T),
        "dgpd": np.ascontiguousarray(dgp.astype(FP8NP)),
        "gmat": ct((np.arange(128)[:, None] // 8 == np.arange(16)[None, :])),
        "gmt": ct((np.arange(16)[:, None] == np.arange(128)[None, :] // 8)),
    }
    in_maps = []
    for i in range(NCORES):
        sl = slice(i * BLOC, (i + 1) * BLOC)
        m = dict(common)
        m["x4"] = ct(x[sl].reshape(BLOC, CH, 128, HW))
        m["wt"] = ct(w[sl].reshape(BLOC, 4, 128).transpose(2, 0, 1))
        in_maps.append(m)
    return in_maps


LAST_EXEC_NS = None


def _run(inputs, trace=False):
    global LAST_EXEC_NS
    nc = _get_nc()
    in_maps = _prep_maps(**inputs)
    res = run_bass_kernel_spmd(nc, in_maps, core_ids=list(range(NCORES)), trace=trace)
    LAST_EXEC_NS = res.exec_time_ns
    outs = [res.results[i]["out4"].reshape(BLOC, C, H, W) for i in range(NCORES)]
    return np.concatenate(outs, axis=0).astype(np.float32)


def kernel(**inputs):
    return _run({k: np.asarray(v) for k, v in inputs.items()}, trace=False)
